# revision 12
# baseline (speedup 1.0000x reference)
"""PowerSpectrumModel Trainium2 kernel v2 (8 NeuronCores, SPMD).

Strategy (data-parallel over atoms, structures disjoint per shard):
 - Host: cut the atom axis at structure boundaries into 8 balanced shards;
   quantize ps to fp8 twice (e4m3 for the DoubleRow MLP matmuls, e3m4 for
   the precision-sensitive linear psl branch), pre-transpose both into
   feature-major tile layout so the device does only contiguous DMA;
   fold the x32 weight-quantization scales into the SiLU activations and
   cancel the coherent W2-quantization error with a host-computed bias.
 - Device, per 512-atom tile:
     psT4/psT3 <- contiguous loads                              [DMA]
     h1   = W1 @ psT4 via 8 DoubleRow matmuls (K=256 each)      [PE]
     sil1 = silu(h1/32) -> e4m3                                 [ACT]
     h2   = W2 @ sil1 via 2 DoubleRow matmuls                   [PE]
     sil2 = silu(h2/32 + c2) -> fp16                            [ACT]
     psl  = Wpsl @ psT3 (8 col-tiled M=1 matmuls, e3m4)         [PE]
     psnn = wout @ sil2 (2 col-tiled M=1 matmuls, fp16)         [PE]
     partial rows + host-exact species row -> columnize (N=1),
     windowed one-hot segment matmul (N=32) accumulates all of
     this core's per-structure energies into a [1,256] PSUM row. [PE/DVE]
 - Host: slice per-core structure ranges, concat -> [2000, 1].
"""

import numpy as np
import ml_dtypes

N_ATOMS = 200000
N_FEAT = 1024
N_SPECIES = 4
N_STRUCT = 2000
H1 = 256
H2 = 256
SCALE = 1.0
N_CORES = 8
TILE = 512
CHUNK = 128
SMAX = 256  # per-core structure capacity (PSUM row)
S = 32.0    # weight quantization scale

E4 = ml_dtypes.float8_e4m3
E3 = ml_dtypes.float8_e3m4

_BUILD_CACHE = {}
TRACE = False
LAST_EXEC_NS = None
LAST_RESULTS = None


def _split_waits(nc, mybir, maxw=1):
    """walrus on this build rejects >1 sync wait per instruction; move
    overflow waits onto preceding same-engine NoOps."""
    cnt = 0
    for f in nc.m.functions:
        for blk in f.blocks:
            if not hasattr(blk, "instructions"):
                continue
            out = []
            changed = False
            for inst in blk.instructions:
                si = getattr(inst, "sync_info", None)
                if si is not None and si.on_wait and len(si.on_wait) > maxw:
                    waits = list(si.on_wait)
                    keep = waits[-maxw:]
                    extra = waits[:-maxw]
                    while extra:
                        chunk, extra = extra[:maxw], extra[maxw:]
                        cnt += 1
                        out.append(
                            mybir.InstNoOp(
                                name=f"waitfix-{cnt}",
                                engine=inst.engine,
                                text_hint="waitfix",
                                bass_nofuse=True,
                                ins=[],
                                outs=[],
                                sync_info=mybir.SyncInfo(on_wait=chunk, on_update=[]),
                            )
                        )
                    si.on_wait = keep
                    changed = True
                out.append(inst)
            if changed:
                blk.instructions[:] = out
    return cnt


def _build(Ta, C, W, base, split_waits=True, act=None):
    import concourse.bass as bass
    import concourse.tile as tile
    import concourse.mybir as mybir
    from contextlib import ExitStack

    f16 = mybir.dt.float16
    f32 = mybir.dt.float32
    f8e4 = mybir.dt.float8e4
    f8e3 = mybir.dt.float8e3
    AF = mybir.ActivationFunctionType
    ACT = AF.Silu if act is None else getattr(AF, act)
    ALU = mybir.AluOpType
    DR = mybir.MatmulPerfMode.DoubleRow
    DRS = mybir.MatmulPerfMode.DoubleRowSwInterleave
    PSUM = bass.MemorySpace.PSUM
    nT = Ta // TILE

    nc = bass.Bass("TRN2", target_bir_lowering=False, debug=False)

    ps4_d = nc.dram_tensor("ps4", [nT * 128, 8, TILE], f8e4, kind="ExternalInput").ap()
    ps3_d = nc.dram_tensor("ps3", [nT * 128, 8, TILE], f8e3, kind="ExternalInput").ap()
    w1_d = nc.dram_tensor("w1", [128, 4, 2, 2, 128], f8e4, kind="ExternalInput").ap()
    w2_d = nc.dram_tensor("w2", [128, 2, 2, 128], f8e4, kind="ExternalInput").ap()
    wpsl_d = nc.dram_tensor("wpsl", [128, 8], f8e3, kind="ExternalInput").ap()
    wout_d = nc.dram_tensor("wout", [128, 2, 16], f8e4, kind="ExternalInput").ap()
    c2b_d = nc.dram_tensor("c2b", [128, 2], f32, kind="ExternalInput").ap()
    comp_d = nc.dram_tensor("comp", [CHUNK, C], f32, kind="ExternalInput").ap()
    oh_d = nc.dram_tensor("oh", [CHUNK, C, W], f16, kind="ExternalInput").ap()
    out_d = nc.dram_tensor("out", [128, 2], f32, kind="ExternalOutput").ap()

    with tile.TileContext(nc) as tc, ExitStack() as ctx:
        const = ctx.enter_context(tc.tile_pool(name="const", bufs=1))
        p4 = ctx.enter_context(tc.tile_pool(name="p4", bufs=8))
        p3 = ctx.enter_context(tc.tile_pool(name="p3", bufs=8))
        silp = ctx.enter_context(tc.tile_pool(name="sil", bufs=2))
        rowp = ctx.enter_context(tc.tile_pool(name="row", bufs=3))
        pp_h1 = ctx.enter_context(tc.tile_pool(name="pph1", bufs=1, space=PSUM))
        pp_h2 = ctx.enter_context(tc.tile_pool(name="pph2", bufs=1, space=PSUM))
        pp_e = ctx.enter_context(tc.tile_pool(name="ppe", bufs=1, space=PSUM))
        pp_ec = ctx.enter_context(tc.tile_pool(name="ppec", bufs=2, space=PSUM))
        pp_seg = ctx.enter_context(tc.tile_pool(name="ppseg", bufs=1, space=PSUM))

        # ---- constants (w1 DMA first; the rest are issued after the
        # first ps tile loads so tile-0 compute starts ~2.5us in) ----
        w1_sb = const.tile([128, 4, 2, 2, 128], f8e4, tag="w1")
        nc.sync.dma_start(w1_sb[:], w1_d[:])
        w2_sb = const.tile([128, 2, 2, 128], f8e4, tag="w2")
        wpsl_sb = const.tile([128, 8], f8e3, tag="wpsl")
        # [128, 2, 16] with the weight in col 0: the dual-fp8 ldweights
        # path needs a 16-byte-aligned stride between the two k-tile columns.
        wout_sb = const.tile([128, 2, 16], f8e4, tag="wout")
        c2b_sb = const.tile([128, 2], f32, tag="c2b")
        comp_sb = const.tile([CHUNK, C], f32, tag="comp")
        oh_sb = const.tile([CHUNK, C, W], f16, tag="oh")
        ones_sb = const.tile([97, 1], f16, tag="ones")
        nc.gpsimd.memset(ones_sb[:], 1.0)
        ones32_sb = const.tile([97, 1], f32, tag="ones32")
        nc.gpsimd.memset(ones32_sb[:], 1.0)
        zrow_sb = const.tile([128, SMAX], f16, tag="zrow")
        nc.gpsimd.memset(zrow_sb[:], 0.0)

        # seg_ps rows 0/32/64/96 accumulate the 4 chunks of each tile as
        # concurrent col-tiled matmuls; open/close the psum group over the
        # whole tile with zero-product matmuls.
        seg_ps = pp_seg.tile([128, SMAX], f32, tag="seg")
        nc.tensor.matmul(
            seg_ps[:], zrow_sb[:, 0:128], zrow_sb[:], start=True, stop=False,
            skip_group_check=True,
        )
        # e-partials bank: psl col-groups write rows 0/32/64/96, psnn rows
        # 0/32; rows between stay 0 from this one-time clear, so a K=97
        # ones-matmul sums the partials.
        e_ps = pp_e.tile([128, TILE], f32, tag="e")
        nc.vector.memset(e_ps[:], 0.0)

        inv_s = 1.0 / S

        big4s = {}

        def _fetch4(tt):
            if tt not in big4s and tt < nT:
                b4 = p4.tile([128, 8, TILE], f8e4, tag="psT4", name=f"psT4_{tt}")
                nc.sync.dma_start(b4[:], ps4_d[tt * 128 : (tt + 1) * 128])
                big4s[tt] = b4
            return big4s.get(tt)

        for t in range(nT):
            big4 = _fetch4(t)
            _fetch4(t + 1)
            big3 = p3.tile([128, 8, TILE], f8e3, tag="psT3", name=f"psT3_{t}")
            nc.sync.dma_start(big3[:], ps3_d[t * 128 : (t + 1) * 128])
            if t == 0:
                nc.sync.dma_start(w2_sb[:], w2_d[:])
                nc.sync.dma_start(wpsl_sb[:], wpsl_d[:])
                nc.sync.dma_start(wout_sb[:], wout_d[:])
                nc.sync.dma_start(c2b_sb[:], c2b_d[:])
                nc.sync.dma_start(comp_sb[:], comp_d[:])
                nc.sync.dma_start(oh_sb[:], oh_d[:])

            # ---- h1: 4 DoubleRow k-groups x 2 m-halves
            h1ps = [pp_h1.tile([128, TILE], f32, tag=f"h1m{m}", name=f"h1ps{t}_{m}") for m in range(2)]
            for g in range(4):
                for m in range(2):
                    nc.tensor.matmul(
                        h1ps[m][:],
                        w1_sb[:, g, m],
                        big4[:, 2 * g : 2 * g + 2, :],
                        start=(g == 0),
                        stop=(g == 3),
                        perf_mode=DRS,
                    )

            # ---- psl: 8 col-tiled M=1 matmuls (e3m4), partials on rows
            # 0/32/64/96 of e_ps (rows 32/64/96 end here; row 0 at psnn).
            # high_priority keeps the block contiguous in the PE queue so the
            # 4-way col-tiled waves co-issue instead of being preempted by
            # the previous tile's epilogue matmuls.
            with tc.high_priority(offset=48):
                for k in range(8):
                    g = 32 * (k % 4)
                    nc.tensor.matmul(
                        e_ps[g : g + 1, :],
                        wpsl_sb[:, k : k + 1],
                        big3[:, k, :],
                        start=(k < 4),
                        stop=(k >= 5),
                        tile_position=(0, g),
                    )

            sil1 = silp.tile([128, 2, TILE], f8e4, tag="sil1")
            for m in range(2):
                nc.scalar.activation(sil1[:, m, :], h1ps[m][:], ACT, scale=inv_s)

            # ---- h2: 1 DoubleRow k-group x 2 m-halves
            h2ps = [pp_h2.tile([128, TILE], f32, tag=f"h2m{m}", name=f"h2ps{t}_{m}") for m in range(2)]
            for m in range(2):
                nc.tensor.matmul(
                    h2ps[m][:],
                    w2_sb[:, m],
                    sil1[:, :, :],
                    start=True,
                    stop=True,
                    perf_mode=DRS,
                )
            sil2 = silp.tile([128, 2, TILE], f8e4, tag="sil2")
            for m in range(2):
                nc.scalar.activation(
                    sil2[:, m, :], h2ps[m][:], ACT,
                    scale=inv_s, bias=c2b_sb[:, m : m + 1],
                )

            # ---- psnn: one DoubleRow M=1 matmul (K=256) onto e_ps row 0,
            # deprioritized so it embeds into the next tile's big-matmul run
            _pn = tc.cur_priority
            tc.cur_priority = _pn + 45
            nc.tensor.matmul(
                e_ps[0:1, :],
                wout_sb[:, :, 0:1],
                sil2[:, :, :],
                start=False,
                stop=True,
                perf_mode=DR,
                tile_position=(0, 0),
            )
            tc.cur_priority = _pn + 1

            # partial rows -> SBUF in one copy
            e_row = rowp.tile([97, TILE], f16, tag="erow")
            nc.vector.tensor_copy(e_row[:], e_ps[0:97, :])

            # ---- column-ize the 4 chunks into one [128,4] psum tile,
            # add host-exact species energy in a single op, then 4
            # co-issued windowed one-hot segment matmuls
            ec_ps = pp_ec.tile([128, 4], f32, tag="ec")
            for cc in range(4):
                nc.tensor.matmul(
                    ec_ps[:, cc : cc + 1],
                    e_row[0:97, cc * 128 : (cc + 1) * 128],
                    ones_sb[:],
                    start=True,
                    stop=True,
                )
            e_col = rowp.tile([128, 4], f16, tag="ecol")
            nc.vector.tensor_add(e_col[:], ec_ps[:], comp_sb[:, 4 * t : 4 * t + 4])
            # deprioritize the seg quartet so it joins the next tile's
            # small-matmul block instead of splitting the big-matmul run
            _p = tc.cur_priority
            tc.cur_priority = _p + 40
            for cc in range(4):
                ch = t * 4 + cc
                b = base[ch]
                g = 32 * cc
                nc.tensor.matmul(
                    seg_ps[g : g + 1, b : b + W],
                    e_col[:, cc : cc + 1],
                    oh_sb[:, ch, :],
                    start=False,
                    stop=False,
                    tile_position=(0, g),
                    skip_group_check=True,
                )
            tc.cur_priority = _p + 4

        nc.tensor.matmul(
            seg_ps[:], zrow_sb[:, 0:128], zrow_sb[:], start=False, stop=True,
            skip_group_check=True,
        )
        # fold the 4 chunk rows and split [SMAX] into two [128] columns
        seg_sb = rowp.tile([97, SMAX], f32, tag="segsb")
        nc.vector.tensor_copy(seg_sb[:], seg_ps[0:97, :])
        out_sb = rowp.tile([128, 2], f32, tag="outsb")
        for j in range(2):
            oc_ps = pp_ec.tile([128, 1], f32, tag="ec", name=f"oc{j}")
            nc.tensor.matmul(
                oc_ps[:],
                seg_sb[0:97, j * 128 : (j + 1) * 128],
                ones32_sb[:],
                start=True,
                stop=True,
            )
            nc.scalar.activation(out_sb[:, j : j + 1], oc_ps[:], AF.Copy, scale=inv_s)
        nc.sync.dma_start(out_d[:], out_sb[:])

    if split_waits:
        _split_waits(nc, mybir)
    return nc


def _install_ntff_hook():
    """Register the axon NTFF profile hook (missing antenv.axon_hooks in
    this image) so run_bass_kernel_spmd(trace=True) can report exec_time_ns."""
    import sys
    import types

    try:
        import antenv.axon_hooks  # noqa: F401

        return
    except ImportError:
        pass
    from trn_agent_boot.trn_boot import _ntff_profile_via_ctypes

    hook = _ntff_profile_via_ctypes("/opt/axon/libaxon_pjrt.so")
    mod = types.ModuleType("antenv.axon_hooks")
    mod.get_axon_ntff_profile_hook = lambda: hook
    mod.set_axon_ntff_profile_hook = lambda h: None
    sys.modules["antenv.axon_hooks"] = mod
    import antenv

    antenv.axon_hooks = mod
    import concourse.bass_utils as bu

    bu.upload_artifacts = lambda tmpdir: tmpdir


def _prep(ps, numbers, batch, W_comp, W_psl, W_h1, W_h2, W_out):
    """All host-side quantization/layout. Returns (shards, Ta, C, W, base,
    in_maps-shared weights, per-shard arrays)."""
    counts = np.bincount(batch, minlength=N_STRUCT)
    cum = np.zeros(N_STRUCT + 1, dtype=np.int64)
    np.cumsum(counts, out=cum[1:])

    s_cut = [i * N_STRUCT // N_CORES for i in range(N_CORES + 1)]
    shards = []
    for i in range(N_CORES):
        s_lo, s_hi = s_cut[i], s_cut[i + 1]
        a_lo, a_hi = int(cum[s_lo]), int(cum[s_hi])
        n_at, n_st = a_hi - a_lo, s_hi - s_lo
        assert n_st <= SMAX, f"shard {i}: {n_st} structs > {SMAX}"
        shards.append((s_lo, s_hi, a_lo, a_hi, n_at, n_st))

    Ta = max(s[4] for s in shards)
    Ta = (Ta + TILE - 1) // TILE * TILE
    nT = Ta // TILE
    C = Ta // CHUNK

    # global quantization (single pass over the big array)
    ps4_all = ps.astype(E4)
    ps3_all = ps.astype(E3)

    # window bases: chunk ch touches a contiguous run of relative structure
    # ids; pick a shared-across-cores window start per chunk.
    relb_list = []
    lo = np.full(C, 10**9, dtype=np.int64)
    hi = np.full(C, -1, dtype=np.int64)
    for s_lo, s_hi, a_lo, a_hi, n_at, n_st in shards:
        rb = np.full(Ta, -1, dtype=np.int64)
        rb[:n_at] = batch[a_lo:a_hi] - s_lo
        relb_list.append(rb)
        rbc = rb.reshape(C, CHUNK)
        valid = rbc >= 0
        anyv = valid.any(axis=1)
        mn = np.where(anyv, np.where(valid, rbc, 10**9).min(axis=1), 10**9)
        mx = np.where(anyv, np.where(valid, rbc, -1).max(axis=1), -1)
        lo = np.minimum(lo, mn)
        hi = np.maximum(hi, mx)

    Wwin = 32
    spread = np.where(hi >= 0, hi - np.minimum(lo, hi) + 1, 1)
    if spread.max() > Wwin:
        Wwin = SMAX  # fallback: full-width windows
    base = np.minimum(np.where(lo > hi, 0, lo), SMAX - Wwin).astype(np.int64)
    base = np.maximum(base, 0)

    # quantized weights (feature-major, DoubleRow pair layout)
    def swi(base):
        # base[g_or_1, i, p, mh, j] -> [p, g, mh, 256] with free bytes
        # [A_{127}, B_{127}, A_{126}, B_{126}, ...] (interleaved pairs,
        # columns reversed) as DoubleRowSwInterleave expects.
        rev = base[:, :, :, :, ::-1]                 # j -> 127-m2
        # -> [p, g, mh, m2, i]
        return np.ascontiguousarray(rev.transpose(2, 0, 3, 4, 1))

    W1s = (S * W_h1.astype(np.float64)).astype(np.float32)  # [256, 1024]
    W1q = W1s.T.astype(E4)                                   # [1024, 256]
    w1 = swi(W1q.reshape(4, 2, 128, 2, 128)).reshape(128, 4, 2, 2, 128)
    W2s = (S * W_h2.astype(np.float64)).astype(np.float32)  # [256, 256]
    W2q = W2s.T.astype(E4)                                   # [256, 256]
    w2 = swi(W2q.reshape(1, 2, 128, 2, 128)).reshape(128, 2, 2, 128)
    wpsl = np.ascontiguousarray(
        (S * W_psl[0].astype(np.float64)).astype(np.float32).astype(E3).reshape(8, 128).T
    )
    Wos = (S * W_out[0].astype(np.float64)).astype(np.float32)  # [256]
    Woq = Wos.astype(E4)
    wout = np.zeros((128, 2, 16), dtype=E4)
    wout[:, :, 0] = Woq.reshape(2, 128).T

    # c2 bias: cancels the coherent part of W2 quantization error; delta:
    # coherent part of wout/sil2 quantization, folded into the species term.
    idx = np.arange(0, ps.shape[0], max(1, ps.shape[0] // 16384))
    smp = ps4_all[idx].astype(np.float32)
    h1s = smp @ W1q.astype(np.float32)
    sil1s = (h1s / S) / (1.0 + np.exp(-(h1s / S)))
    sil1q = sil1s.astype(E4).astype(np.float32)
    mu1 = sil1q.mean(axis=0)
    V2 = W2s.T - W2q.astype(np.float32)   # [256, 256] residual
    c2 = (mu1 @ V2) / S                   # [256]
    c2b = np.ascontiguousarray(c2.reshape(2, 128).T).astype(np.float32)
    h2s = sil1q @ W2q.astype(np.float32)
    x = h2s / S + c2
    sil2s_t = x / (1.0 + np.exp(-x))
    sil2q = sil2s_t.astype(E4).astype(np.float32)
    mu2 = sil2q.mean(axis=0)
    beta2 = (sil2q - sil2s_t).mean(axis=0)
    Woq32 = Woq.astype(np.float32)
    delta = float(((Woq32 - Wos) * mu2).sum() + (Woq32 * beta2).sum())

    comp_tab = (S * W_comp[0][:N_SPECIES].astype(np.float64)).astype(np.float32) - delta

    in_maps = []
    for i, (s_lo, s_hi, a_lo, a_hi, n_at, n_st) in enumerate(shards):
        def tilefy(q_all, dt):
            a = np.zeros((Ta, N_FEAT), dtype=np.uint8)
            a[:n_at] = q_all[a_lo:a_hi].view(np.uint8)
            t = a.reshape(nT, TILE, 8, 128).transpose(0, 3, 2, 1)
            return np.ascontiguousarray(t).reshape(nT * 128, 8, TILE).view(dt)

        rb = relb_list[i]
        rbw = rb.reshape(C, CHUNK) - base[:, None]
        rbw[rb.reshape(C, CHUNK) < 0] = -1000
        # one-hot window masks, precomputed: oh[p, ch, j] = (rbw[ch, p] == j)
        oh = (rbw[:, :, None] == np.arange(Wwin)[None, None, :]).astype(np.float16)
        oh = np.ascontiguousarray(oh.transpose(1, 0, 2))  # [128, C, W]

        comp = np.zeros(Ta, dtype=np.float32)
        comp[:n_at] = comp_tab[numbers[a_lo:a_hi]]
        comp = np.ascontiguousarray(comp.reshape(C, CHUNK).T)

        in_maps.append(
            {
                "ps4": tilefy(ps4_all, E4),
                "ps3": tilefy(ps3_all, E3),
                "w1": w1,
                "w2": w2,
                "wpsl": wpsl,
                "wout": wout,
                "c2b": c2b,
                "comp": comp,
                "oh": oh,
            }
        )

    return shards, counts, Ta, C, Wwin, base, in_maps


def kernel(ps, numbers, batch, W_comp, W_psl, W_h1, W_h2, W_out):
    global LAST_EXEC_NS, LAST_RESULTS
    from concourse.bass_utils import run_bass_kernel_spmd

    if TRACE:
        _install_ntff_hook()

    ps = np.asarray(ps)
    numbers = np.asarray(numbers)
    batch = np.asarray(batch)
    W_comp = np.asarray(W_comp, dtype=np.float32)
    W_psl = np.asarray(W_psl, dtype=np.float32)
    W_h1 = np.asarray(W_h1, dtype=np.float32)
    W_h2 = np.asarray(W_h2, dtype=np.float32)
    W_out = np.asarray(W_out, dtype=np.float32)

    shards, counts, Ta, C, Wwin, base, in_maps = _prep(
        ps, numbers, batch, W_comp, W_psl, W_h1, W_h2, W_out
    )

    key = (Ta, C, Wwin, tuple(base.tolist()))
    if key not in _BUILD_CACHE:
        _BUILD_CACHE.clear()
        _BUILD_CACHE[key] = _build(Ta, C, Wwin, base)
    nc = _BUILD_CACHE[key]

    res = run_bass_kernel_spmd(nc, in_maps, list(range(N_CORES)), trace=TRACE)
    LAST_EXEC_NS = res.exec_time_ns
    LAST_RESULTS = res

    out = np.zeros((N_STRUCT, 1), dtype=np.float32)
    for i, (s_lo, s_hi, a_lo, a_hi, n_at, n_st) in enumerate(shards):
        o = res.results[i]["out"]
        vals = o.T.reshape(-1)[:n_st].astype(np.float32)
        empty = counts[s_lo:s_hi] == 0
        if empty.any():
            vals = np.where(empty, 0.0, vals)
        out[s_lo:s_hi, 0] = vals
    return out


# revision 13
# speedup vs baseline: 1.1272x; 1.1272x over previous
"""PowerSpectrumModel Trainium2 kernel v2 (8 NeuronCores, SPMD).

Strategy (data-parallel over atoms, structures disjoint per shard):
 - Host: cut the atom axis at structure boundaries into 8 balanced shards;
   quantize ps to fp8 twice (e4m3 for the DoubleRow MLP matmuls, e3m4 for
   the precision-sensitive linear psl branch), pre-transpose both into
   feature-major tile layout so the device does only contiguous DMA;
   fold the x32 weight-quantization scales into the SiLU activations and
   cancel the coherent W2-quantization error with a host-computed bias.
 - Device, per 512-atom tile:
     psT4/psT3 <- contiguous loads                              [DMA]
     h1   = W1 @ psT4 via 8 DoubleRow matmuls (K=256 each)      [PE]
     sil1 = silu(h1/32) -> e4m3                                 [ACT]
     h2   = W2 @ sil1 via 2 DoubleRow matmuls                   [PE]
     sil2 = silu(h2/32 + c2) -> fp16                            [ACT]
     psl  = Wpsl @ psT3 (8 col-tiled M=1 matmuls, e3m4)         [PE]
     psnn = wout @ sil2 (2 col-tiled M=1 matmuls, fp16)         [PE]
     partial rows + host-exact species row -> columnize (N=1),
     windowed one-hot segment matmul (N=32) accumulates all of
     this core's per-structure energies into a [1,256] PSUM row. [PE/DVE]
 - Host: slice per-core structure ranges, concat -> [2000, 1].
"""

import numpy as np
import ml_dtypes

N_ATOMS = 200000
N_FEAT = 1024
N_SPECIES = 4
N_STRUCT = 2000
H1 = 256
H2 = 256
SCALE = 1.0
N_CORES = 8
TILE = 512
CHUNK = 128
SMAX = 256  # per-core structure capacity (PSUM row)
S = 32.0    # weight quantization scale

E4 = ml_dtypes.float8_e4m3
E3 = ml_dtypes.float8_e3m4

_BUILD_CACHE = {}
TRACE = False
LAST_EXEC_NS = None
LAST_RESULTS = None


def _split_waits(nc, mybir, maxw=1):
    """walrus on this build rejects >1 sync wait per instruction; move
    overflow waits onto preceding same-engine NoOps."""
    cnt = 0
    for f in nc.m.functions:
        for blk in f.blocks:
            if not hasattr(blk, "instructions"):
                continue
            out = []
            changed = False
            for inst in blk.instructions:
                si = getattr(inst, "sync_info", None)
                if si is not None and si.on_wait and len(si.on_wait) > maxw:
                    waits = list(si.on_wait)
                    keep = waits[-maxw:]
                    extra = waits[:-maxw]
                    while extra:
                        chunk, extra = extra[:maxw], extra[maxw:]
                        cnt += 1
                        out.append(
                            mybir.InstNoOp(
                                name=f"waitfix-{cnt}",
                                engine=inst.engine,
                                text_hint="waitfix",
                                bass_nofuse=True,
                                ins=[],
                                outs=[],
                                sync_info=mybir.SyncInfo(on_wait=chunk, on_update=[]),
                            )
                        )
                    si.on_wait = keep
                    changed = True
                out.append(inst)
            if changed:
                blk.instructions[:] = out
    return cnt


def _build(Ta, C, W, base, split_waits=True, act=None):
    import concourse.bass as bass
    import concourse.tile as tile
    import concourse.mybir as mybir
    from contextlib import ExitStack

    f16 = mybir.dt.float16
    f32 = mybir.dt.float32
    f8e4 = mybir.dt.float8e4
    f8e3 = mybir.dt.float8e3
    AF = mybir.ActivationFunctionType
    ACT = AF.Silu if act is None else getattr(AF, act)
    ALU = mybir.AluOpType
    DR = mybir.MatmulPerfMode.DoubleRow
    DRS = mybir.MatmulPerfMode.DoubleRowSwInterleave
    PSUM = bass.MemorySpace.PSUM
    nT = Ta // TILE

    nc = bass.Bass("TRN2", target_bir_lowering=False, debug=False)

    ps4_d = nc.dram_tensor("ps4", [nT * 128, 8, TILE], f8e4, kind="ExternalInput").ap()
    ps3_d = nc.dram_tensor("ps3", [nT * 128, 8, TILE], f8e3, kind="ExternalInput").ap()
    w1_d = nc.dram_tensor("w1", [128, 4, 2, 2, 128], f8e4, kind="ExternalInput").ap()
    w2_d = nc.dram_tensor("w2", [128, 2, 2, 128], f8e4, kind="ExternalInput").ap()
    wpsl_d = nc.dram_tensor("wpsl", [128, 8], f8e3, kind="ExternalInput").ap()
    wout_d = nc.dram_tensor("wout", [128, 2, 16], f8e4, kind="ExternalInput").ap()
    c2b_d = nc.dram_tensor("c2b", [128, 2], f32, kind="ExternalInput").ap()
    comp_d = nc.dram_tensor("comp", [CHUNK, C], f32, kind="ExternalInput").ap()
    oh_d = nc.dram_tensor("oh", [CHUNK, C, W], f16, kind="ExternalInput").ap()
    out_d = nc.dram_tensor("out", [128, 2], f32, kind="ExternalOutput").ap()

    with tile.TileContext(nc) as tc, ExitStack() as ctx:
        const = ctx.enter_context(tc.tile_pool(name="const", bufs=1))
        p4 = ctx.enter_context(tc.tile_pool(name="p4", bufs=6))
        p3 = ctx.enter_context(tc.tile_pool(name="p3", bufs=6))
        silp = ctx.enter_context(tc.tile_pool(name="sil", bufs=2))
        rowp = ctx.enter_context(tc.tile_pool(name="row", bufs=3))
        pp_h1 = ctx.enter_context(tc.tile_pool(name="pph1", bufs=1, space=PSUM))
        pp_h2 = ctx.enter_context(tc.tile_pool(name="pph2", bufs=1, space=PSUM))
        pp_e = ctx.enter_context(tc.tile_pool(name="ppe", bufs=1, space=PSUM))
        pp_ec = ctx.enter_context(tc.tile_pool(name="ppec", bufs=2, space=PSUM))
        pp_seg = ctx.enter_context(tc.tile_pool(name="ppseg", bufs=1, space=PSUM))

        # ---- constants (w1 DMA first; the rest are issued after the
        # first ps tile loads so tile-0 compute starts ~2.5us in) ----
        w1_sb = const.tile([128, 4, 2, 2, 128], f8e4, tag="w1")
        nc.sync.dma_start(w1_sb[:], w1_d[:])
        w2_sb = const.tile([128, 2, 2, 128], f8e4, tag="w2")
        wpsl_sb = const.tile([128, 8], f8e3, tag="wpsl")
        # [128, 2, 16] with the weight in col 0: the dual-fp8 ldweights
        # path needs a 16-byte-aligned stride between the two k-tile columns.
        wout_sb = const.tile([128, 2, 16], f8e4, tag="wout")
        c2b_sb = const.tile([128, 2], f32, tag="c2b")
        comp_sb = const.tile([CHUNK, C], f32, tag="comp")
        oh_sb = const.tile([CHUNK, C, W], f16, tag="oh")
        ones_sb = const.tile([97, 1], f16, tag="ones")
        nc.gpsimd.memset(ones_sb[:], 1.0)
        ones32_sb = const.tile([97, 1], f32, tag="ones32")
        nc.gpsimd.memset(ones32_sb[:], 1.0)
        zrow_sb = const.tile([128, SMAX], f16, tag="zrow")
        nc.gpsimd.memset(zrow_sb[:], 0.0)

        # seg_ps rows 0/32/64/96 accumulate the 4 chunks of each tile as
        # concurrent col-tiled matmuls; open/close the psum group over the
        # whole tile with zero-product matmuls.
        seg_ps = pp_seg.tile([128, SMAX], f32, tag="seg")
        nc.tensor.matmul(
            seg_ps[:], zrow_sb[:, 0:128], zrow_sb[:], start=True, stop=False,
            skip_group_check=True,
        )
        # e-partials bank: psl col-groups write rows 0/32/64/96, psnn rows
        # 0/32; rows between stay 0 from this one-time clear, so a K=97
        # ones-matmul sums the partials.
        e_ps = pp_e.tile([128, TILE], f32, tag="e")
        nc.vector.memset(e_ps[:], 0.0)

        inv_s = 1.0 / S

        big4s = {}

        def _fetch4(tt):
            if tt not in big4s and tt < nT:
                b4 = p4.tile([128, 8, TILE], f8e4, tag="psT4", name=f"psT4_{tt}")
                nc.sync.dma_start(b4[:], ps4_d[tt * 128 : (tt + 1) * 128])
                big4s[tt] = b4
            return big4s.get(tt)

        for t in range(nT):
            big4 = _fetch4(t)
            _fetch4(t + 1)
            big3 = p3.tile([128, 8, TILE], f8e3, tag="psT3", name=f"psT3_{t}")
            nc.sync.dma_start(big3[:], ps3_d[t * 128 : (t + 1) * 128])
            if t == 0:
                nc.sync.dma_start(w2_sb[:], w2_d[:])
                nc.sync.dma_start(wpsl_sb[:], wpsl_d[:])
                nc.sync.dma_start(wout_sb[:], wout_d[:])
                nc.sync.dma_start(c2b_sb[:], c2b_d[:])
                nc.sync.dma_start(comp_sb[:], comp_d[:])
                nc.sync.dma_start(oh_sb[:], oh_d[:])

            # ---- h1: 4 DoubleRow k-groups x 2 m-halves
            h1ps = [pp_h1.tile([128, TILE], f32, tag=f"h1m{m}", name=f"h1ps{t}_{m}") for m in range(2)]
            for g in range(4):
                for m in range(2):
                    nc.tensor.matmul(
                        h1ps[m][:],
                        w1_sb[:, g, m],
                        big4[:, 2 * g : 2 * g + 2, :],
                        start=(g == 0),
                        stop=(g == 3),
                        perf_mode=DRS,
                    )

            # ---- psl: 8 col-tiled M=1 matmuls (e3m4), partials on rows
            # 0/32/64/96 of e_ps (rows 32/64/96 end here; row 0 at psnn).
            # high_priority keeps the block contiguous in the PE queue so the
            # 4-way col-tiled waves co-issue instead of being preempted by
            # the previous tile's epilogue matmuls.
            with tc.high_priority(offset=48):
                for k in range(8):
                    g = 32 * (k % 4)
                    nc.tensor.matmul(
                        e_ps[g : g + 1, :],
                        wpsl_sb[:, k : k + 1],
                        big3[:, k, :],
                        start=(k < 4),
                        stop=(k >= 5),
                        tile_position=(0, g),
                    )

            sil1 = silp.tile([128, 2, TILE], f8e4, tag="sil1")
            for m in range(2):
                nc.scalar.activation(sil1[:, m, :], h1ps[m][:], ACT, scale=inv_s)

            # ---- h2: 1 DoubleRow k-group x 2 m-halves
            h2ps = [pp_h2.tile([128, TILE], f32, tag=f"h2m{m}", name=f"h2ps{t}_{m}") for m in range(2)]
            for m in range(2):
                nc.tensor.matmul(
                    h2ps[m][:],
                    w2_sb[:, m],
                    sil1[:, :, :],
                    start=True,
                    stop=True,
                    perf_mode=DRS,
                )
            sil2 = silp.tile([128, 2, TILE], f8e4, tag="sil2")
            for m in range(2):
                nc.scalar.activation(
                    sil2[:, m, :], h2ps[m][:], ACT,
                    scale=inv_s, bias=c2b_sb[:, m : m + 1],
                )

            # ---- psnn: one DoubleRow M=1 matmul (K=256) onto e_ps row 0,
            # deprioritized so it embeds into the next tile's big-matmul run
            _pn = tc.cur_priority
            tc.cur_priority = _pn + 12
            nc.tensor.matmul(
                e_ps[0:1, :],
                wout_sb[:, :, 0:1],
                sil2[:, :, :],
                start=False,
                stop=True,
                perf_mode=DR,
                tile_position=(0, 0),
            )
            tc.cur_priority = _pn + 1

            # partial rows -> SBUF in one copy
            e_row = rowp.tile([97, TILE], f16, tag="erow")
            nc.vector.tensor_copy(e_row[:], e_ps[0:97, :])

            # ---- column-ize the 4 chunks into one [128,4] psum tile,
            # add host-exact species energy in a single op, then 4
            # co-issued windowed one-hot segment matmuls
            ec_ps = pp_ec.tile([128, 4], f32, tag="ec")
            for cc in range(4):
                nc.tensor.matmul(
                    ec_ps[:, cc : cc + 1],
                    e_row[0:97, cc * 128 : (cc + 1) * 128],
                    ones_sb[:],
                    start=True,
                    stop=True,
                )
            e_col = rowp.tile([128, 4], f16, tag="ecol")
            nc.vector.tensor_add(e_col[:], ec_ps[:], comp_sb[:, 4 * t : 4 * t + 4])
            # deprioritize the seg quartet so it joins the next tile's
            # small-matmul block instead of splitting the big-matmul run
            _p = tc.cur_priority
            tc.cur_priority = _p + 40
            for cc in range(4):
                ch = t * 4 + cc
                b = base[ch]
                g = 32 * cc
                nc.tensor.matmul(
                    seg_ps[g : g + 1, b : b + W],
                    e_col[:, cc : cc + 1],
                    oh_sb[:, ch, :],
                    start=False,
                    stop=False,
                    tile_position=(0, g),
                    skip_group_check=True,
                )
            tc.cur_priority = _p + 4

        nc.tensor.matmul(
            seg_ps[:], zrow_sb[:, 0:128], zrow_sb[:], start=False, stop=True,
            skip_group_check=True,
        )
        # fold the 4 chunk rows and split [SMAX] into two [128] columns
        seg_sb = rowp.tile([97, SMAX], f32, tag="segsb")
        nc.vector.tensor_copy(seg_sb[:], seg_ps[0:97, :])
        out_sb = rowp.tile([128, 2], f32, tag="outsb")
        for j in range(2):
            oc_ps = pp_ec.tile([128, 1], f32, tag="ec", name=f"oc{j}")
            nc.tensor.matmul(
                oc_ps[:],
                seg_sb[0:97, j * 128 : (j + 1) * 128],
                ones32_sb[:],
                start=True,
                stop=True,
            )
            nc.scalar.activation(out_sb[:, j : j + 1], oc_ps[:], AF.Copy, scale=inv_s)
        nc.sync.dma_start(out_d[:], out_sb[:])

    if split_waits:
        _split_waits(nc, mybir)
    return nc


def _install_ntff_hook():
    """Register the axon NTFF profile hook (missing antenv.axon_hooks in
    this image) so run_bass_kernel_spmd(trace=True) can report exec_time_ns."""
    import sys
    import types

    try:
        import antenv.axon_hooks  # noqa: F401

        return
    except ImportError:
        pass
    from trn_agent_boot.trn_boot import _ntff_profile_via_ctypes

    hook = _ntff_profile_via_ctypes("/opt/axon/libaxon_pjrt.so")
    mod = types.ModuleType("antenv.axon_hooks")
    mod.get_axon_ntff_profile_hook = lambda: hook
    mod.set_axon_ntff_profile_hook = lambda h: None
    sys.modules["antenv.axon_hooks"] = mod
    import antenv

    antenv.axon_hooks = mod
    import concourse.bass_utils as bu

    bu.upload_artifacts = lambda tmpdir: tmpdir


def _prep(ps, numbers, batch, W_comp, W_psl, W_h1, W_h2, W_out):
    """All host-side quantization/layout. Returns (shards, Ta, C, W, base,
    in_maps-shared weights, per-shard arrays)."""
    counts = np.bincount(batch, minlength=N_STRUCT)
    cum = np.zeros(N_STRUCT + 1, dtype=np.int64)
    np.cumsum(counts, out=cum[1:])

    s_cut = [i * N_STRUCT // N_CORES for i in range(N_CORES + 1)]
    shards = []
    for i in range(N_CORES):
        s_lo, s_hi = s_cut[i], s_cut[i + 1]
        a_lo, a_hi = int(cum[s_lo]), int(cum[s_hi])
        n_at, n_st = a_hi - a_lo, s_hi - s_lo
        assert n_st <= SMAX, f"shard {i}: {n_st} structs > {SMAX}"
        shards.append((s_lo, s_hi, a_lo, a_hi, n_at, n_st))

    Ta = max(s[4] for s in shards)
    Ta = (Ta + TILE - 1) // TILE * TILE
    nT = Ta // TILE
    C = Ta // CHUNK

    # global quantization (single pass over the big array)
    ps4_all = ps.astype(E4)
    ps3_all = ps.astype(E3)

    # window bases: chunk ch touches a contiguous run of relative structure
    # ids; pick a shared-across-cores window start per chunk.
    relb_list = []
    lo = np.full(C, 10**9, dtype=np.int64)
    hi = np.full(C, -1, dtype=np.int64)
    for s_lo, s_hi, a_lo, a_hi, n_at, n_st in shards:
        rb = np.full(Ta, -1, dtype=np.int64)
        rb[:n_at] = batch[a_lo:a_hi] - s_lo
        relb_list.append(rb)
        rbc = rb.reshape(C, CHUNK)
        valid = rbc >= 0
        anyv = valid.any(axis=1)
        mn = np.where(anyv, np.where(valid, rbc, 10**9).min(axis=1), 10**9)
        mx = np.where(anyv, np.where(valid, rbc, -1).max(axis=1), -1)
        lo = np.minimum(lo, mn)
        hi = np.maximum(hi, mx)

    Wwin = 32
    spread = np.where(hi >= 0, hi - np.minimum(lo, hi) + 1, 1)
    if spread.max() > Wwin:
        Wwin = SMAX  # fallback: full-width windows
    base = np.minimum(np.where(lo > hi, 0, lo), SMAX - Wwin).astype(np.int64)
    base = np.maximum(base, 0)

    # quantized weights (feature-major, DoubleRow pair layout)
    def swi(base):
        # base[g_or_1, i, p, mh, j] -> [p, g, mh, 256] with free bytes
        # [A_{127}, B_{127}, A_{126}, B_{126}, ...] (interleaved pairs,
        # columns reversed) as DoubleRowSwInterleave expects.
        rev = base[:, :, :, :, ::-1]                 # j -> 127-m2
        # -> [p, g, mh, m2, i]
        return np.ascontiguousarray(rev.transpose(2, 0, 3, 4, 1))

    W1s = (S * W_h1.astype(np.float64)).astype(np.float32)  # [256, 1024]
    W1q = W1s.T.astype(E4)                                   # [1024, 256]
    w1 = swi(W1q.reshape(4, 2, 128, 2, 128)).reshape(128, 4, 2, 2, 128)
    W2s = (S * W_h2.astype(np.float64)).astype(np.float32)  # [256, 256]
    W2q = W2s.T.astype(E4)                                   # [256, 256]
    w2 = swi(W2q.reshape(1, 2, 128, 2, 128)).reshape(128, 2, 2, 128)
    wpsl = np.ascontiguousarray(
        (S * W_psl[0].astype(np.float64)).astype(np.float32).astype(E3).reshape(8, 128).T
    )
    Wos = (S * W_out[0].astype(np.float64)).astype(np.float32)  # [256]
    Woq = Wos.astype(E4)
    wout = np.zeros((128, 2, 16), dtype=E4)
    wout[:, :, 0] = Woq.reshape(2, 128).T

    # c2 bias: cancels the coherent part of W2 quantization error; delta:
    # coherent part of wout/sil2 quantization, folded into the species term.
    idx = np.arange(0, ps.shape[0], max(1, ps.shape[0] // 16384))
    smp = ps4_all[idx].astype(np.float32)
    h1s = smp @ W1q.astype(np.float32)
    sil1s = (h1s / S) / (1.0 + np.exp(-(h1s / S)))
    sil1q = sil1s.astype(E4).astype(np.float32)
    mu1 = sil1q.mean(axis=0)
    V2 = W2s.T - W2q.astype(np.float32)   # [256, 256] residual
    c2 = (mu1 @ V2) / S                   # [256]
    c2b = np.ascontiguousarray(c2.reshape(2, 128).T).astype(np.float32)
    h2s = sil1q @ W2q.astype(np.float32)
    x = h2s / S + c2
    sil2s_t = x / (1.0 + np.exp(-x))
    sil2q = sil2s_t.astype(E4).astype(np.float32)
    mu2 = sil2q.mean(axis=0)
    beta2 = (sil2q - sil2s_t).mean(axis=0)
    Woq32 = Woq.astype(np.float32)
    delta = float(((Woq32 - Wos) * mu2).sum() + (Woq32 * beta2).sum())

    comp_tab = (S * W_comp[0][:N_SPECIES].astype(np.float64)).astype(np.float32) - delta

    in_maps = []
    for i, (s_lo, s_hi, a_lo, a_hi, n_at, n_st) in enumerate(shards):
        def tilefy(q_all, dt):
            a = np.zeros((Ta, N_FEAT), dtype=np.uint8)
            a[:n_at] = q_all[a_lo:a_hi].view(np.uint8)
            t = a.reshape(nT, TILE, 8, 128).transpose(0, 3, 2, 1)
            return np.ascontiguousarray(t).reshape(nT * 128, 8, TILE).view(dt)

        rb = relb_list[i]
        rbw = rb.reshape(C, CHUNK) - base[:, None]
        rbw[rb.reshape(C, CHUNK) < 0] = -1000
        # one-hot window masks, precomputed: oh[p, ch, j] = (rbw[ch, p] == j)
        oh = (rbw[:, :, None] == np.arange(Wwin)[None, None, :]).astype(np.float16)
        oh = np.ascontiguousarray(oh.transpose(1, 0, 2))  # [128, C, W]

        comp = np.zeros(Ta, dtype=np.float32)
        comp[:n_at] = comp_tab[numbers[a_lo:a_hi]]
        comp = np.ascontiguousarray(comp.reshape(C, CHUNK).T)

        in_maps.append(
            {
                "ps4": tilefy(ps4_all, E4),
                "ps3": tilefy(ps3_all, E3),
                "w1": w1,
                "w2": w2,
                "wpsl": wpsl,
                "wout": wout,
                "c2b": c2b,
                "comp": comp,
                "oh": oh,
            }
        )

    return shards, counts, Ta, C, Wwin, base, in_maps


def kernel(ps, numbers, batch, W_comp, W_psl, W_h1, W_h2, W_out):
    global LAST_EXEC_NS, LAST_RESULTS
    from concourse.bass_utils import run_bass_kernel_spmd

    if TRACE:
        _install_ntff_hook()

    ps = np.asarray(ps)
    numbers = np.asarray(numbers)
    batch = np.asarray(batch)
    W_comp = np.asarray(W_comp, dtype=np.float32)
    W_psl = np.asarray(W_psl, dtype=np.float32)
    W_h1 = np.asarray(W_h1, dtype=np.float32)
    W_h2 = np.asarray(W_h2, dtype=np.float32)
    W_out = np.asarray(W_out, dtype=np.float32)

    shards, counts, Ta, C, Wwin, base, in_maps = _prep(
        ps, numbers, batch, W_comp, W_psl, W_h1, W_h2, W_out
    )

    key = (Ta, C, Wwin, tuple(base.tolist()))
    if key not in _BUILD_CACHE:
        _BUILD_CACHE.clear()
        _BUILD_CACHE[key] = _build(Ta, C, Wwin, base)
    nc = _BUILD_CACHE[key]

    res = run_bass_kernel_spmd(nc, in_maps, list(range(N_CORES)), trace=TRACE)
    LAST_EXEC_NS = res.exec_time_ns
    LAST_RESULTS = res

    out = np.zeros((N_STRUCT, 1), dtype=np.float32)
    for i, (s_lo, s_hi, a_lo, a_hi, n_at, n_st) in enumerate(shards):
        o = res.results[i]["out"]
        vals = o.T.reshape(-1)[:n_st].astype(np.float32)
        empty = counts[s_lo:s_hi] == 0
        if empty.any():
            vals = np.where(empty, 0.0, vals)
        out[s_lo:s_hi, 0] = vals
    return out
